# revision 1
# baseline (speedup 1.0000x reference)
"""Trainium2 Bass kernel for RecursiveMamba130M.

Math: the complex SSM state never needs materializing. With
  R = cos(theta) + j sin(theta),  Bc = Br + j Bi,  Cc = Cr + j Ci,
the per-loop output collapses to
  y_i[t, f] = sum_{k<=i} G_{i-k}[f] * u_k[t, f],   u_k = h_k @ W_in^T
where G_m[f] = sum_s Re(Cc * R^m * Bc)
            = sum_s (CrBr - CiBi) cos(m th) - (CrBi + CiBr) sin(m th).

Sharding: fully data-parallel over the 1024 sequence positions
(128 tokens per core, no collectives); small weights replicated.

Per-core device program (tokens on partitions, fp32/fp32r):
  loop i in 0..3:
    hT   = PE-transpose(h)                  (6x 128x128)
    u    = h @ W_in^T                       (PE, fp32r, N=512 tiles)
    y    = G0*u + acc_i ; acc_j += G_{j-i}*u  (DVE/Pool, G broadcast tiles)
    yT   = PE-transpose(y)                  (12x 128x128)
    z    = y @ out_proj^T                   (PE, fp32r)
    out  = rmsnorm(z); w = h + out; x' = rmsnorm(w); h = x' + step_emb[i+1]
  (norm sums via ACT Square+accum and the identity
   sum w^2 = rs_z^2*sum z^2 + 2 rs_z*sum z*h + sum h^2)
"""

import numpy as np

import concourse.bass as bass
import concourse.tile as tile
from concourse.bacc import Bacc
from concourse import masks, mybir
from concourse.bass_utils import run_bass_kernel_spmd

T = 128          # tokens per core
D = 768          # d_model
F = 1536         # 2 * d_model
NL = 4           # reasoning loops
NCORES = 8
EPS = 1e-6

f32 = mybir.dt.float32
f32r = mybir.dt.float32r
AL = mybir.AluOpType
AF = mybir.ActivationFunctionType

_CACHE = {}


def build_nc():
    nc = Bacc()
    x_d = nc.dram_tensor("x_in", [T, D], f32, kind="ExternalInput")
    winT_d = nc.dram_tensor("winT", [D, F], f32, kind="ExternalInput")
    woutT_d = nc.dram_tensor("woutT", [F, D], f32, kind="ExternalInput")
    g4_d = nc.dram_tensor("g4", [NL, F], f32, kind="ExternalInput")
    s4_d = nc.dram_tensor("s4", [NL, D], f32, kind="ExternalInput")
    out_d = nc.dram_tensor("x_out", [T, D], f32, kind="ExternalOutput")

    with tile.TileContext(nc) as tc:
        with (
            tc.tile_pool(name="wpool", bufs=1) as wpool,
            tc.tile_pool(name="apool", bufs=1) as apool,
            tc.tile_pool(name="work", bufs=2) as work,
            tc.tile_pool(name="scal", bufs=1) as scal,
            tc.tile_pool(name="ps_t", bufs=1, space="PSUM") as ps_t,
            tc.tile_pool(name="ps_u", bufs=1, space="PSUM") as ps_u,
            tc.tile_pool(name="ps_z", bufs=1, space="PSUM") as ps_z,
        ):
            # ---------- constants / weights ----------
            ident = wpool.tile([128, 128], f32, tag="ident")
            masks.make_identity(nc, ident[:])
            ones1 = wpool.tile([1, 128], f32r, tag="ones1")
            nc.vector.memset(ones1[:].bitcast(mybir.dt.uint32), 0x3F800000)
            eps_t = wpool.tile([T, 1], f32, tag="eps_t")
            nc.vector.memset(eps_t[:], EPS)

            x_sb = wpool.tile([T, D], f32, tag="x_sb")
            nc.sync.dma_start(x_sb[:], x_d[:, :])

            winT_sb = []
            for k in range(6):
                wt = wpool.tile([128, F], f32r, tag=f"winT{k}")
                nc.sync.dma_start(wt[:], winT_d[128 * k:128 * (k + 1), :].bitcast(f32r))
                winT_sb.append(wt)

            # step_emb broadcast tiles [128, D] via K=1 matmul
            Sb = []
            for i in range(NL):
                sr = work.tile([1, D], f32r, tag="s_row", bufs=2, name=f"s_row{i}")
                nc.sync.dma_start(sr[:], s4_d[i:i + 1, :].bitcast(f32r))
                sb_ps = ps_z.tile([T, D], f32, tag="z")
                for off, nn in ((0, 512), (512, 256)):
                    nc.tensor.matmul(
                        sb_ps[:, off:off + nn],
                        ones1[:, :],
                        sr[:, off:off + nn],
                        start=True, stop=True,
                    )
                sb = wpool.tile([T, D], f32, tag=f"Sb{i}")
                nc.scalar.copy(sb[:], sb_ps[:])
                Sb.append(sb)

            # G broadcast tiles [128, F]
            Gb = []
            for m in range(NL):
                gr = work.tile([1, F], f32r, tag="g_row", bufs=2, name=f"g_row{m}")
                nc.sync.dma_start(gr[:], g4_d[m:m + 1, :].bitcast(f32r))
                gb_ps = ps_u.tile([T, F], f32, tag="u")
                for n in range(3):
                    nc.tensor.matmul(
                        gb_ps[:, 512 * n:512 * (n + 1)],
                        ones1[:, :],
                        gr[:, 512 * n:512 * (n + 1)],
                        start=True, stop=True,
                    )
                gb = wpool.tile([T, F], f32, tag=f"Gb{m}")
                nc.scalar.copy(gb[:], gb_ps[:])
                Gb.append(gb)

            woutT_sb = []
            for c in range(12):
                wt = wpool.tile([128, D], f32r, tag=f"woutT{c}")
                nc.sync.dma_start(wt[:], woutT_d[128 * c:128 * (c + 1), :].bitcast(f32r))
                woutT_sb.append(wt)

            # ---------- h0 = x + Sb0 ----------
            h = work.tile([T, D], f32, tag="h", bufs=2)
            nc.vector.tensor_add(h[:], x_sb[:], Sb[0][:])

            accs = {}
            for j in (1, 2, 3):
                accs[j] = apool.tile([T, F], f32, tag=f"acc{j}", name=f"acc{j}")

            # ---------- main loop ----------
            for i in range(NL):
                # hT (stationary for MM1)
                hT_ps = ps_t.tile([T, D], f32, tag="t")
                for k in range(6):
                    nc.tensor.transpose(
                        hT_ps[:, 128 * k:128 * (k + 1)],
                        h[:, 128 * k:128 * (k + 1)],
                        ident[:],
                    )
                hT_sb = work.tile([T, D], f32r, tag="hT_sb", bufs=1)
                nc.scalar.copy(hT_sb[:], hT_ps[:])

                # MM1: u = h @ W_in^T   [T, F]
                u_ps = ps_u.tile([T, F], f32, tag="u")
                for k in range(6):
                    for n in range(3):
                        nc.tensor.matmul(
                            u_ps[:, 512 * n:512 * (n + 1)],
                            hT_sb[:, 128 * k:128 * (k + 1)],
                            winT_sb[k][:, 512 * n:512 * (n + 1)],
                            start=(k == 0), stop=(k == 5),
                        )

                # combine: y = G0*u (+ acc_i)
                y = work.tile([T, F], f32, tag="y", bufs=1)
                if i == 0:
                    for n in range(3):
                        sl = slice(512 * n, 512 * (n + 1))
                        nc.vector.tensor_mul(y[:, sl], u_ps[:, sl], Gb[0][:, sl])
                else:
                    for n in range(3):
                        sl = slice(512 * n, 512 * (n + 1))
                        nc.vector.tensor_mul(y[:, sl], u_ps[:, sl], Gb[0][:, sl])
                        nc.vector.tensor_add(y[:, sl], y[:, sl], accs[i][:, sl])

                # acc updates (off critical path): acc_j += G_{j-i} * u
                for j in range(i + 1, NL):
                    m = j - i
                    if i == 0:
                        nc.vector.tensor_mul(accs[j][:], u_ps[:], Gb[m][:])
                    else:
                        tmp_a = work.tile([T, F], f32, tag="tmp_a", bufs=2)
                        nc.vector.tensor_mul(tmp_a[:], u_ps[:], Gb[m][:])
                        nc.gpsimd.tensor_add(accs[j][:], accs[j][:], tmp_a[:])

                # yT (stationary for MM2)
                yT_ps = ps_t.tile([T, F], f32, tag="t")
                for c in range(12):
                    nc.tensor.transpose(
                        yT_ps[:, 128 * c:128 * (c + 1)],
                        y[:, 128 * c:128 * (c + 1)],
                        ident[:],
                    )
                yT_sb = work.tile([T, F], f32r, tag="yT_sb", bufs=1)
                for n in range(3):
                    sl = slice(512 * n, 512 * (n + 1))
                    nc.scalar.copy(yT_sb[:, sl], yT_ps[:, sl])

                # MM2: z = y @ out_proj^T   [T, D]
                z_ps = ps_z.tile([T, D], f32, tag="z")
                for c in range(12):
                    for off, nn in ((0, 512), (512, 256)):
                        nc.tensor.matmul(
                            z_ps[:, off:off + nn],
                            yT_sb[:, 128 * c:128 * (c + 1)],
                            woutT_sb[c][:, off:off + nn],
                            start=(c == 0), stop=(c == 11),
                        )

                # mixer rmsnorm + residual + loop rmsnorm
                ss_z = scal.tile([T, 1], f32, tag="ss_z")
                sq_scr = work.tile([T, D], f32, tag="scr", bufs=2)
                nc.scalar.activation(sq_scr[:], z_ps[:], AF.Square, accum_out=ss_z[:])
                sq_z = scal.tile([T, 1], f32, tag="sq_z")
                nc.scalar.activation(sq_z[:], ss_z[:], AF.Sqrt,
                                     bias=eps_t[:, :], scale=1.0 / D)
                rs_z = scal.tile([T, 1], f32, tag="rs_z")
                nc.vector.reciprocal(rs_z[:], sq_z[:])

                # w = z * rs_z + h
                w = work.tile([T, D], f32, tag="w", bufs=1)
                nc.vector.scalar_tensor_tensor(
                    out=w[:], in0=z_ps[:], scalar=rs_z[:], in1=h[:],
                    op0=AL.mult, op1=AL.add,
                )

                ss_w = scal.tile([T, 1], f32, tag="ss_w")
                sq_scr2 = work.tile([T, D], f32, tag="scr", bufs=2)
                nc.scalar.activation(sq_scr2[:], w[:], AF.Square, accum_out=ss_w[:])
                sq_w = scal.tile([T, 1], f32, tag="sq_w")
                nc.scalar.activation(sq_w[:], ss_w[:], AF.Sqrt,
                                     bias=eps_t[:, :], scale=1.0 / D)
                rs_w = scal.tile([T, 1], f32, tag="rs_w")
                nc.vector.reciprocal(rs_w[:], sq_w[:])

                if i < NL - 1:
                    h_next = work.tile([T, D], f32, tag="h", bufs=2)
                    nc.vector.scalar_tensor_tensor(
                        out=h_next[:], in0=w[:], scalar=rs_w[:], in1=Sb[i + 1][:],
                        op0=AL.mult, op1=AL.add,
                    )
                    h = h_next
                else:
                    nc.vector.tensor_scalar_mul(w[:], w[:], rs_w[:, :])
                    nc.sync.dma_start(out_d[:, :], w[:])

    nc.compile()
    return nc


def _host_prep(x, in_proj_base, lora_A, lora_B, A_theta, B_real, B_imag,
               C_real, C_imag, out_proj_w, step_emb):
    W_in = in_proj_base.astype(np.float64) + 2.0 * (
        lora_B.astype(np.float64) @ lora_A.astype(np.float64))
    winT = np.ascontiguousarray(W_in.T).astype(np.float32)
    woutT = np.ascontiguousarray(out_proj_w.T).astype(np.float32)

    th = A_theta.astype(np.float64)
    P = (C_real.astype(np.float64) * B_real.astype(np.float64)
         - C_imag.astype(np.float64) * B_imag.astype(np.float64))
    Q = (C_real.astype(np.float64) * B_imag.astype(np.float64)
         + C_imag.astype(np.float64) * B_real.astype(np.float64))
    g4 = np.stack([
        (P * np.cos(m * th) - Q * np.sin(m * th)).sum(-1).reshape(-1)
        for m in range(NL)
    ]).astype(np.float32)                                   # [4, 1536]
    s4 = np.ascontiguousarray(step_emb).astype(np.float32)  # [4, 768]
    return winT, woutT, g4, s4


def kernel(x, in_proj_base, lora_A, lora_B, A_theta, B_real, B_imag,
           C_real, C_imag, out_proj_w, mixer_norm_w, loop_norm_w, step_emb,
           _trace=False):
    x = np.asarray(x, dtype=np.float32)
    winT, woutT, g4, s4 = _host_prep(
        np.asarray(x), np.asarray(in_proj_base), np.asarray(lora_A),
        np.asarray(lora_B), np.asarray(A_theta), np.asarray(B_real),
        np.asarray(B_imag), np.asarray(C_real), np.asarray(C_imag),
        np.asarray(out_proj_w), np.asarray(step_emb))
    # mixer_norm_w / loop_norm_w are ones per the problem spec; rmsnorm weight
    # multiplies are identity and omitted on device.

    if "nc" not in _CACHE:
        _CACHE["nc"] = build_nc()
    nc = _CACHE["nc"]

    shared = {"winT": winT, "woutT": woutT, "g4": g4, "s4": s4}
    in_maps = [
        {**shared, "x_in": np.ascontiguousarray(x[0, T * c:T * (c + 1), :])}
        for c in range(NCORES)
    ]
    res = run_bass_kernel_spmd(nc, in_maps, list(range(NCORES)), trace=_trace)
    out = np.concatenate(
        [np.asarray(res.results[c]["x_out"]) for c in range(NCORES)], axis=0)
    if _trace:
        _CACHE["last_result"] = res
    return out[None, :, :].astype(np.float32)

